# revision 1
# baseline (speedup 1.0000x reference)
"""Trainium2 Bass kernel for BioSphericalCKN1D.

  out[b,l,f] = s * dot(x[b,l:l+7,:], k[:,:,f]) / sqrt(sum(x[b,l:l+7,:]^2)+eps) + b[f]

Strategy (per core, pure batch data-parallel: 8 batches/core):
  * Host packs x into a 4-phase "transposed polyphase" layout:
      x4[b, p*20+c, t] = x[b, 4t+p, c]   -> [8, 80, T+4] (T=L/4, zero padded)
    so the conv becomes matmuls with contraction over the partition dim.
  * Position l = 4t+q. Window tap k gives source phase column t+j where
    j=(q+k)//4 in {0,1,2}. j=2 has only 3 (p,q) combos (source p in {0,1});
    those 40 rows are replicated on-chip (SBUF->SBUF DMA, shifted by 2
    columns) into partitions 80..119 so the whole j in {0,2} contribution is
    ONE 120-row matmul; j=1 is an 80-row matmul at column offset +1.
  * Windowed sum-of-squares uses the same two matmuls on x^2 with 0/1
    stationary masks, with the result broadcast across the 32 filters by
    making the mask stationary dense over (q,f) -> rsqrt input lands as
    [128, N] directly in PSUM.
  * rsqrt via exp(-0.5*ln(ssq+eps)) on ScalarE (Rsqrt/Reciprocal activations
    are banned for accuracy; Ln+Exp share one table set).
  * Epilogue: VectorE multiply (PSUM dot x SBUF rsq), GpSimdE per-partition
    bias add, DMA out. Host unpacks the polyphase output layout.

  Measured on HW (reps-in-NEFF delta timing): ~320 us/core-iteration with
  2048-col blocks (vs ~505 us at 1024-col), rel err vs fp32 reference
  9.5e-6. Per-instruction sync overhead dominates on this stack, so wider
  blocks (fewer instructions) win. (float32r matmul variant: similar speed
  at rel err 1.5e-4 — not worth the accuracy cost.)
"""

import os
import sys

import numpy as np

for _p in ("/opt/trn_rl_repo",):
    if _p not in sys.path and os.path.isdir(_p):
        sys.path.insert(0, _p)

import concourse.bacc as bacc
import concourse.bass as bass
import concourse.mybir as mybir
import concourse.tile as tile
from concourse.bass_utils import run_bass_kernel_spmd

B, L, C, F, KT = 64, 16384, 20, 32, 7
NCORES = 8
NB = B // NCORES  # batches per core
PH = 4  # phases
T = L // PH  # 4096
NT = 512  # matmul moving free dim
PAIR = 2 * NT  # epilogue batching unit (2 PSUM banks)
EPS = 1e-7

_F32 = mybir.dt.float32
_F32R = mybir.dt.float32r  # single-pass PE fp32 (TF32-like rounding), 4x faster


def _pin_act_tables():
    """Force Ln/Exp onto the one table set containing both, so the ACT table
    is loaded once instead of ping-ponging every iteration. Preserves set
    indices (walrus act_func_set_id is positional)."""
    import concourse.hw_specs as hw_specs

    real = hw_specs.get_activation_tables
    AFT = mybir.ActivationFunctionType

    def patched(arch):
        tabs = {k: set(v) for k, v in real(arch).items()}
        for name, fns in tabs.items():
            if name != "natural_log_exp_and_others":
                fns.discard(AFT.Ln)
                fns.discard(AFT.Exp)
        return tabs

    bacc.get_activation_tables = patched


def _build_weight_mats(kk: np.ndarray, s: float):
    """Stationary matrices [row=(p,c) (+ext rows), col=(q,f)].

    A (120 rows): j=0 taps (rows 0..79) + j=2 taps on the replicated
    shifted rows 80..119 (source phase p' in {0,1}).
    Bm (80 rows): j=1 taps, applied at moving-column offset +1.
    As/Bs: same sparsity masks with 1.0 entries (windowed sum of squares).
    """
    A = np.zeros((120, 128), np.float32)
    As = np.zeros((120, 128), np.float32)
    Bm = np.zeros((80, 128), np.float32)
    Bs = np.zeros((80, 128), np.float32)
    for p in range(PH):
        for q in range(PH):
            t0 = p - q  # j=0 tap
            if 0 <= t0 <= KT - 1:
                A[p * C:(p + 1) * C, q * F:(q + 1) * F] = s * kk[t0]
                As[p * C:(p + 1) * C, q * F:(q + 1) * F] = 1.0
            t1 = PH + p - q  # j=1 tap
            if 0 <= t1 <= KT - 1:
                Bm[p * C:(p + 1) * C, q * F:(q + 1) * F] = s * kk[t1]
                Bs[p * C:(p + 1) * C, q * F:(q + 1) * F] = 1.0
    for p2 in range(2):  # j=2 tap, on ext rows 80..119
        for q in range(PH):
            t2 = 2 * PH + p2 - q
            if 0 <= t2 <= KT - 1:
                A[80 + p2 * C:80 + (p2 + 1) * C, q * F:(q + 1) * F] = s * kk[t2]
                As[80 + p2 * C:80 + (p2 + 1) * C, q * F:(q + 1) * F] = 1.0
    return A, Bm, As, Bs


def build_nc(nb: int = NB, t_dim: int = T, nt: int = NT, reps: int = 1, xdt=None, pin_tables: bool = False) -> bass.Bass:
    if xdt is None:
        xdt = _F32
    pair = 2 * nt
    tpad = t_dim + PH
    npairs = t_dim // pair
    assert t_dim % pair == 0

    if pin_tables:
        _pin_act_tables()
    nc = bacc.Bacc()
    x4 = nc.declare_dram_parameter("x4", [nb, 80, tpad], xdt, isOutput=False)
    adot = nc.declare_dram_parameter("adot", [120, 128], xdt, isOutput=False)
    bdot = nc.declare_dram_parameter("bdot", [80, 128], xdt, isOutput=False)
    assq = nc.declare_dram_parameter("assq", [120, 128], xdt, isOutput=False)
    bssq = nc.declare_dram_parameter("bssq", [80, 128], xdt, isOutput=False)
    bvec = nc.declare_dram_parameter("bvec", [128, 1], _F32, isOutput=False)
    out4 = nc.declare_dram_parameter("out", [nb, 128, t_dim], _F32, isOutput=True)

    AFT = mybir.ActivationFunctionType

    with tile.TileContext(nc) as tc:
        with (
            tc.tile_pool(name="wts", bufs=1) as wpool,
            tc.tile_pool(name="xin", bufs=3) as xpool,
            tc.tile_pool(name="xsq", bufs=2) as qpool,
            tc.tile_pool(name="vec", bufs=2) as vpool,
            tc.tile_pool(name="obuf", bufs=2) as opool,
            tc.tile_pool(name="psa", bufs=1, space=bass.MemorySpace.PSUM) as pspool_a,
            tc.tile_pool(name="psb", bufs=1, space=bass.MemorySpace.PSUM) as pspool_b,
        ):
            a_t = wpool.tile([120, 128], xdt)
            nc.sync.dma_start(a_t[:, :], adot[:, :])
            b_t = wpool.tile([80, 128], xdt)
            nc.sync.dma_start(b_t[:, :], bdot[:, :])
            as_t = wpool.tile([120, 128], xdt)
            nc.sync.dma_start(as_t[:, :], assq[:, :])
            bs_t = wpool.tile([80, 128], xdt)
            nc.sync.dma_start(bs_t[:, :], bssq[:, :])
            bv_t = wpool.tile([128, 1], _F32)
            nc.sync.dma_start(bv_t[:, :], bvec[:, :])
            eps_t = wpool.tile([128, 1], _F32)
            nc.gpsimd.memset(eps_t[:, :], EPS)

            for _rep in range(reps):
              for bi in range(nb):
                for qi in range(t_dim // (4 * nt)):
                    q0 = qi * 4 * nt
                    quad = 4 * nt
                    xin = xpool.tile([120, quad + 4], xdt)
                    nc.sync.dma_start(xin[0:80, :], x4[bi, :, q0:q0 + quad + 4])
                    # j=2 source rows (phases 0,1) shifted 2 phase-columns
                    nc.sync.dma_start(xin[80:120, 0:quad + 2], xin[0:40, 2:quad + 4])

                    xsq = qpool.tile([120, quad + 2], xdt)
                    nc.vector.tensor_mul(
                        xsq[:, :], xin[0:120, 0:quad + 2], xin[0:120, 0:quad + 2]
                    )

                    ps_a = pspool_a.tile([128, quad], _F32)
                    ps_b = pspool_b.tile([128, quad], _F32)
                    for h in range(4):
                        o = h * nt
                        nc.tensor.matmul(
                            ps_a[:, o:o + nt], a_t[:, :],
                            xin[0:120, o:o + nt], start=True, stop=False,
                        )
                        nc.tensor.matmul(
                            ps_a[:, o:o + nt], b_t[:, :],
                            xin[0:80, o + 1:o + 1 + nt], start=False, stop=True,
                        )
                        nc.tensor.matmul(
                            ps_b[:, o:o + nt], as_t[:, :],
                            xsq[0:120, o:o + nt], start=True, stop=False,
                        )
                        nc.tensor.matmul(
                            ps_b[:, o:o + nt], bs_t[:, :],
                            xsq[0:80, o + 1:o + 1 + nt], start=False, stop=True,
                        )

                    tln = vpool.tile([128, quad], _F32)
                    nc.scalar.activation(
                        tln[:, :], ps_b[:, :], AFT.Ln, bias=eps_t[:, 0:1], scale=1.0
                    )
                    rsq = vpool.tile([128, quad], _F32)
                    nc.scalar.activation(rsq[:, :], tln[:, :], AFT.Exp, bias=0.0, scale=-0.5)
                    tmp = opool.tile([128, quad], _F32)
                    nc.vector.tensor_mul(tmp[:, :], ps_a[:, :], rsq[:, :])
                    osb = opool.tile([128, quad], _F32)
                    nc.gpsimd.tensor_scalar_add(osb[:, :], tmp[:, :], bv_t[:, 0:1])
                    nc.sync.dma_start(out4[bi, :, q0:q0 + quad], osb[:, :])

    nc.finalize()
    return nc


def pack_x(xc: np.ndarray) -> np.ndarray:
    """[nb, L', C] -> polyphase-transposed [nb, 80, L'/4 + 4] (zero padded)."""
    nb, lc, cc = xc.shape
    t = lc // PH
    xr = xc.reshape(nb, t, PH, cc).transpose(0, 2, 3, 1).reshape(nb, PH * cc, t)
    return np.concatenate(
        [xr, np.zeros((nb, PH * cc, PH), np.float32)], axis=2
    ).copy()


def unpack_out(r: np.ndarray, lc: int) -> np.ndarray:
    """[nb, 128, T'] -> [nb, L'-6, F]."""
    nb, _, t = r.shape
    y = r.reshape(nb, PH, F, t).transpose(0, 3, 1, 2).reshape(nb, PH * t, F)
    return y[:, :lc - KT + 1, :]


_NC_CACHE: dict = {}


def _get_nc() -> bass.Bass:
    if "nc" not in _NC_CACHE:
        _NC_CACHE["nc"] = build_nc()
    return _NC_CACHE["nc"]


def make_in_maps(x, k, s, b, np_xdt=np.float32):
    x = np.ascontiguousarray(np.asarray(x, dtype=np.float32))
    kk = np.asarray(k, dtype=np.float32)
    sv = float(np.asarray(s).reshape(-1)[0])
    bb = np.asarray(b, dtype=np.float32)

    a_m, b_m, as_m, bs_m = _build_weight_mats(kk, sv)
    bvec = np.ascontiguousarray(np.tile(bb, PH).reshape(128, 1).astype(np.float32))

    in_maps = []
    for ci in range(NCORES):
        xc = x[ci * NB:(ci + 1) * NB]
        in_maps.append(
            {
                "x4": pack_x(xc).astype(np_xdt),
                "adot": a_m.astype(np_xdt),
                "bdot": b_m.astype(np_xdt),
                "assq": as_m.astype(np_xdt),
                "bssq": bs_m.astype(np_xdt),
                "bvec": bvec,
            }
        )
    return in_maps


def run(x, k, s, b, trace: bool = False):
    nc = _get_nc()
    in_maps = make_in_maps(x, k, s, b)
    res = run_bass_kernel_spmd(nc, in_maps, list(range(NCORES)), trace=trace)
    outs = [unpack_out(np.asarray(res.results[ci]["out"]), L) for ci in range(NCORES)]
    return np.concatenate(outs, axis=0), res


def kernel(**inputs) -> np.ndarray:
    out, _ = run(inputs["x"], inputs["k"], inputs["s"], inputs["b"])
    return out



# revision 3
# speedup vs baseline: 2.6797x; 2.6797x over previous
"""Trainium2 Bass kernel for BioSphericalCKN1D — v2.6.

  out[b,l,f] = s * dot(x[b,l:l+7,:], k[:,:,f]) / sqrt(sum(x[b,l:l+7,:]^2)+eps) + b[f]

Strategy (vs the 320us fp32 v1 baseline; rel-err gate is 2e-2 so reduced
precision is free accuracy budget):
  * Polyphase layout (4 phases): conv becomes matmuls contracting over the
    partition dim, output cols = (phase q, filter f) = 128. bf16 x/weights
    -> PE at 1 cycle/row (fp32 was 4), input DMA halved; output DMA'd bf16
    and host-cast to fp32.
  * Windowed sum-of-squares: host precomputes fp8e4(x^2) (no on-chip
    square pass); ssq = fp8 DoubleRow matmuls (2 taps per PE pass, 0.5
    cyc/row). Odd-parity outputs split into two accumulating matmuls so
    every fp8 moving AP starts at an even byte offset (odd bases wedge the
    PE). Even/odd halves land split in PSUM; downstream APs deinterleave.
  * j=2 tap source rows (80:120) replicated on-chip via SBUF->SBUF DMA
    (saves HBM reads), issued from the Act engine's HWDGE.
  * PSUM double-buffered (1024-col blocks, (2+2) banks x2 bufs) so PE
    overlaps the epilogue instead of serializing against it.
  * Epilogue in 3 instructions: Act Sqrt (ssq+eps -> n, PSUM->SBUF) and 2
    custom DVE ops out = (dot + bias*n) * recip_approx(n) = dot/n + bias
    (BITWISE_NOT reciprocal seed + 1 Newton, 8/8 DVE stages; bias rides
    the per-partition scalar slot; rank-3 strided in0/out APs deinterleave
    the half-layout). Recip approx max rel err ~1.7e-3.

Per-core engine budget (8 batches, 32768 phase-cols): PE 37.6us,
DVE 34.1us, Act 27.3us, DMA 16.3MB HBM ~45us. Measured end-to-end rel
err 7.3e-3.
"""

import os
import re
import sys

import numpy as np

for _p in ("/opt/trn_rl_repo",):
    if _p not in sys.path and os.path.isdir(_p):
        sys.path.insert(0, _p)

import concourse.bacc as bacc
import concourse.bass as bass
import concourse.mybir as mybir
import concourse.tile as tile
from concourse.bass_utils import run_bass_kernel_spmd

import ml_dtypes

B, L, C, F, KT = 64, 16384, 20, 32, 7
NCORES = 8
NB = B // NCORES  # batches per core
PH = 4  # phases
T = L // PH  # 4096
NT = 512  # matmul moving free dim (one PSUM bank)
BLK = 1024  # epilogue block (2 PSUM banks)
EPS = 1e-7

_F32 = mybir.dt.float32
_BF16 = mybir.dt.bfloat16
_FP8 = mybir.dt.float8e4
_NPBF16 = ml_dtypes.bfloat16
_NPFP8 = mybir.dt.np(mybir.dt.float8e4)

# ---------------------------------------------------------------------------
# Custom DVE op: out = (Src0 + C0*Src1) * recip_approx(Src1)
#   Src0 = dot (PSUM fp32), Src1 = n = sqrt(ssq+eps), C0 = per-partition bias,
#   C1/C2 = reciprocal seed/Newton constants.
# recip_approx: y0 = bitcast(~bits(n)) * C1 ; y1 = y0 * (C2 - n*y0)
# ---------------------------------------------------------------------------
import concourse.dve_ops as dve_ops_mod
from concourse.dve_spec import Spec, lower, Src0, Src1, C0, C1, C2, AluOp, Bin

RECIP_C1 = -0.23549792
RECIP_C2 = 2.0017324


def _ref_mul_recip_bias(in0, in1, c0, c1, c2):
    not_n = (~in1.view(np.int32)).view(np.float32)
    y0 = not_n * c1
    y1 = y0 * (c2 - in1 * y0)
    return (in0 + c0 * in1) * y1


def _make_fused_op():
    name = "MUL_RECIP_BIAS_ANT_X"
    for op in dve_ops_mod.OPS:
        if op.name == name:
            return op
    _not_n = Bin(AluOp.BITWISE_NOT, Src1, Src1)
    _y0 = _not_n * C1
    _y1 = _y0 * (C2 - Src1 * _y0)
    body = (Src0 + C0 * Src1) * _y1
    op = dve_ops_mod.DveOp(
        name, Spec(body=body, reference=_ref_mul_recip_bias), subdim=False,
        uops_sha={},
    )
    # register in the module tables used by table-gen + IR trace
    dve_ops_mod.OPS.append(op)
    dve_ops_mod.CUSTOM_DVE_SPECS[name] = op.spec
    dve_ops_mod._SUB_OPCODE_FOR_NAME[name] = (
        dve_ops_mod._CUSTOM_DVE_ROW_BASE + len(dve_ops_mod.OPS) - 1
    )
    assert dve_ops_mod._SUB_OPCODE_FOR_NAME[name] < 0x20
    # pin the uops sha (compile() raises with the computed sha otherwise)
    for ver in ("v3", "v4"):
        try:
            op.compile(ver)
        except ValueError as e:
            m = re.search(r'"%s"\]="([0-9a-f]+)"' % ver, str(e))
            assert m, f"cannot pin sha for {ver}: {e}"
            op.uops_sha[ver] = m.group(1)
            op.compile(ver)
    return op


MUL_RECIP_BIAS = _make_fused_op()


def _build_weight_mats(kk: np.ndarray, s: float):
    """Stationary matrices [row=(p,c) (+ext rows), col=(q,f)].

    A (120 rows): j=0 taps (rows 0..79) + j=2 taps on the replicated
    shifted rows 80..119 (source phase p' in {0,1}).
    Bm (80 rows): j=1 taps, applied at moving-column offset +1.
    As/Bs: same sparsity masks with 1.0 entries (windowed sum of squares).
    """
    A = np.zeros((120, 128), np.float32)
    As = np.zeros((120, 128), np.float32)
    Bm = np.zeros((80, 128), np.float32)
    Bs = np.zeros((80, 128), np.float32)
    for p in range(PH):
        for q in range(PH):
            t0 = p - q  # j=0 tap
            if 0 <= t0 <= KT - 1:
                A[p * C:(p + 1) * C, q * F:(q + 1) * F] = s * kk[t0]
                As[p * C:(p + 1) * C, q * F:(q + 1) * F] = 1.0
            t1 = PH + p - q  # j=1 tap
            if 0 <= t1 <= KT - 1:
                Bm[p * C:(p + 1) * C, q * F:(q + 1) * F] = s * kk[t1]
                Bs[p * C:(p + 1) * C, q * F:(q + 1) * F] = 1.0
    for p2 in range(2):  # j=2 tap, on ext rows 80..119
        for q in range(PH):
            t2 = 2 * PH + p2 - q
            if 0 <= t2 <= KT - 1:
                A[80 + p2 * C:80 + (p2 + 1) * C, q * F:(q + 1) * F] = s * kk[t2]
                As[80 + p2 * C:80 + (p2 + 1) * C, q * F:(q + 1) * F] = 1.0
    # DoubleRow ssq mask halves [120, 256], pair-slot layout (d m):
    #   even-t outputs: one matmul, [As | Bs], moving base t0 (slots = cols
    #     t, t+1).
    #   odd-t outputs: fp8 moving APs must start at even byte offsets, so
    #     split into [0 | As] @ base t0 (slot1 = col t+... = tap 0) and
    #     [Bs | 0] @ base t0+2 (slot0 = tap +1), accumulated.
    Ws = np.zeros((120, 256), np.float32)
    Ws[:, 0:128] = As
    Ws[0:80, 128:256] = Bs
    Woa = np.zeros((120, 256), np.float32)
    Woa[:, 128:256] = As
    Wob = np.zeros((120, 256), np.float32)
    Wob[0:80, 0:128] = Bs
    return A, Bm, Ws, Woa, Wob


def build_nc(nb: int = NB, t_dim: int = T, reps: int = 1) -> bass.Bass:
    tpad = t_dim + PH
    nblk = t_dim // BLK
    assert t_dim % BLK == 0

    nc = bacc.Bacc()
    x4 = nc.declare_dram_parameter("x4", [nb, 80, tpad], _BF16, isOutput=False)
    xq8 = nc.declare_dram_parameter("xq8", [nb, 80, tpad], _FP8, isOutput=False)
    adot = nc.declare_dram_parameter("adot", [120, 128], _BF16, isOutput=False)
    bdot = nc.declare_dram_parameter("bdot", [80, 128], _BF16, isOutput=False)
    wssq = nc.declare_dram_parameter("wssq", [120, 256], _FP8, isOutput=False)
    wsoa = nc.declare_dram_parameter("wsoa", [120, 256], _FP8, isOutput=False)
    wsob = nc.declare_dram_parameter("wsob", [120, 256], _FP8, isOutput=False)
    bvec = nc.declare_dram_parameter("bvec", [128, 1], _F32, isOutput=False)
    out4 = nc.declare_dram_parameter("out", [nb, 128, t_dim], _BF16, isOutput=True)

    AFT = mybir.ActivationFunctionType

    with tile.TileContext(nc) as tc:
        with (
            tc.tile_pool(name="wts", bufs=1) as wpool,
            tc.tile_pool(name="xin", bufs=3) as xpool,
            tc.tile_pool(name="xsq", bufs=2) as qpool,
            tc.tile_pool(name="nrm", bufs=2) as vpool,
            tc.tile_pool(name="obuf", bufs=2) as opool,
            tc.tile_pool(name="psa", bufs=2, space=bass.MemorySpace.PSUM) as pspool_a,
            tc.tile_pool(name="psb", bufs=2, space=bass.MemorySpace.PSUM) as pspool_b,
        ):
            a_t = wpool.tile([120, 128], _BF16)
            nc.sync.dma_start(a_t[:, :], adot[:, :])
            b_t = wpool.tile([80, 128], _BF16)
            nc.sync.dma_start(b_t[:, :], bdot[:, :])
            ws_t = wpool.tile([120, 256], _FP8)
            nc.sync.dma_start(ws_t[:, :], wssq[:, :])
            woa_t = wpool.tile([120, 256], _FP8)
            nc.sync.dma_start(woa_t[:, :], wsoa[:, :])
            wob_t = wpool.tile([120, 256], _FP8)
            nc.sync.dma_start(wob_t[:, :], wsob[:, :])
            bv_t = wpool.tile([128, 1], _F32)
            nc.sync.dma_start(bv_t[:, :], bvec[:, :])
            eps_t = wpool.tile([128, 1], _F32)
            nc.gpsimd.memset(eps_t[:, :], EPS)

            for _rep in range(reps):
              for bi in range(nb):
                for qi in range(nblk):
                    q0 = qi * BLK
                    # rows 80:120 (j=2 tap sources, shifted 2 cols) are
                    # replicated on-chip: SBUF->SBUF DMA, saving HBM reads.
                    xin = xpool.tile([120, BLK + 4], _BF16)
                    nc.sync.dma_start(xin[0:80, :], x4[bi, :, q0:q0 + BLK + 4])
                    nc.scalar.dma_start(
                        xin[80:120, 0:BLK + 2], xin[0:40, 2:BLK + 4]
                    )
                    xq = qpool.tile([120, BLK + 4], _FP8)
                    nc.sync.dma_start(xq[0:80, :], xq8[bi, :, q0:q0 + BLK + 4])
                    nc.scalar.dma_start(
                        xq[80:120, 0:BLK + 2], xq[0:40, 2:BLK + 4]
                    )

                    ps_a = pspool_a.tile([128, BLK], _F32)
                    ps_b = pspool_b.tile([128, BLK], _F32)
                    for h in range(BLK // NT):
                        o = h * NT
                        nc.tensor.matmul(
                            ps_a[:, o:o + NT], a_t[:, :],
                            xin[0:120, o:o + NT], start=True, stop=False,
                        )
                        nc.tensor.matmul(
                            ps_a[:, o:o + NT], b_t[:, :],
                            xin[0:80, o + 1:o + 1 + NT], start=False, stop=True,
                        )
                        # ssq via fp8 DoubleRow: out[m,n] = sum_d W[:,d,:].T @
                        # X[:,d,n], X pairs = adjacent cols. Even-t and odd-t
                        # outputs land in the two 256-halves of [o, o+512);
                        # all moving bases even (odd bases wedge the PE).
                        mv0 = xq[0:120, o:o + NT].rearrange("k (n d) -> k d n", d=2)
                        nc.tensor.matmul(
                            ps_b[:, o:o + NT // 2],
                            ws_t[:, :].rearrange("k (d m) -> k d m", d=2),
                            mv0, start=True, stop=True,
                            perf_mode=mybir.MatmulPerfMode.DoubleRow,
                        )
                        nc.tensor.matmul(
                            ps_b[:, o + NT // 2:o + NT],
                            woa_t[:, :].rearrange("k (d m) -> k d m", d=2),
                            mv0, start=True, stop=False,
                            perf_mode=mybir.MatmulPerfMode.DoubleRow,
                        )
                        nc.tensor.matmul(
                            ps_b[:, o + NT // 2:o + NT],
                            wob_t[:, :].rearrange("k (d m) -> k d m", d=2),
                            xq[0:120, o + 2:o + 2 + NT].rearrange(
                                "k (n d) -> k d n", d=2
                            ),
                            start=False, stop=True,
                            perf_mode=mybir.MatmulPerfMode.DoubleRow,
                        )

                    # nf inherits ps_b's half-layout; the fused op's strided
                    # rank-3 in0/out APs iterate (d, n) to match it, so osb
                    # comes out in natural t order.
                    nf = vpool.tile([128, BLK], _F32)
                    nc.scalar.activation(
                        nf[:, :], ps_b[:, :], AFT.Sqrt, bias=eps_t[:, 0:1],
                        scale=1.0,
                    )
                    osb = opool.tile([128, BLK], _BF16)
                    for h in range(BLK // NT):
                        o = h * NT
                        nc.vector._custom_dve(
                            MUL_RECIP_BIAS,
                            out=osb[:, o:o + NT].rearrange("p (n d) -> p d n", d=2),
                            in0=ps_a[:, o:o + NT].rearrange("p (n d) -> p d n", d=2),
                            in1=nf[:, o:o + NT],
                            s0=bv_t[:, 0:1],
                            s1=RECIP_C1,
                            imm2=RECIP_C2,
                        )
                    nc.sync.dma_start(out4[bi, :, q0:q0 + BLK], osb[:, :])

    nc.finalize()
    return nc


def pack_x(xc: np.ndarray) -> np.ndarray:
    """[nb, L', C] fp32 -> polyphase-transposed [nb, 120, L'/4 + 4] fp32.

    Rows 0..79: x4[b, p*20+c, t] = x[b, 4t+p, c], zero-padded tail.
    Rows 80..119: rows 0..39 shifted left 2 cols (the j=2 tap sources).
    """
    nb, lc, cc = xc.shape
    t = lc // PH
    xr = xc.reshape(nb, t, PH, cc).transpose(0, 2, 3, 1).reshape(nb, PH * cc, t)
    base = np.concatenate(
        [xr, np.zeros((nb, PH * cc, PH), np.float32)], axis=2
    )
    ext = np.zeros((nb, 40, t + PH), np.float32)
    ext[:, :, 0:t + PH - 2] = base[:, 0:40, 2:t + PH]
    return np.concatenate([base, ext], axis=1)


def unpack_out(r: np.ndarray, lc: int) -> np.ndarray:
    """[nb, 128, T'] bf16 -> [nb, L'-6, F] fp32."""
    nb, _, t = r.shape
    y = np.asarray(r).astype(np.float32)
    y = y.reshape(nb, PH, F, t).transpose(0, 3, 1, 2).reshape(nb, PH * t, F)
    return np.ascontiguousarray(y[:, :lc - KT + 1, :])


_NC_CACHE: dict = {}


def _get_nc() -> bass.Bass:
    if "nc" not in _NC_CACHE:
        _NC_CACHE["nc"] = build_nc()
    return _NC_CACHE["nc"]


def make_in_maps(x, k, s, b):
    x = np.ascontiguousarray(np.asarray(x, dtype=np.float32))
    kk = np.asarray(k, dtype=np.float32)
    sv = float(np.asarray(s).reshape(-1)[0])
    bb = np.asarray(b, dtype=np.float32)

    a_m, b_m, ws_m, woa_m, wob_m = _build_weight_mats(kk, sv)
    bvec = np.ascontiguousarray(np.tile(bb, PH).reshape(128, 1).astype(np.float32))

    in_maps = []
    for ci in range(NCORES):
        xc = x[ci * NB:(ci + 1) * NB]
        xp = pack_x(xc)[:, 0:80]
        in_maps.append(
            {
                "x4": xp.astype(_NPBF16),
                "xq8": (xp * xp).astype(_NPFP8),
                "adot": a_m.astype(_NPBF16),
                "bdot": b_m.astype(_NPBF16),
                "wssq": ws_m.astype(_NPFP8),
                "wsoa": woa_m.astype(_NPFP8),
                "wsob": wob_m.astype(_NPFP8),
                "bvec": bvec,
            }
        )
    return in_maps


def run(x, k, s, b, trace: bool = False):
    nc = _get_nc()
    in_maps = make_in_maps(x, k, s, b)
    res = run_bass_kernel_spmd(nc, in_maps, list(range(NCORES)), trace=trace)
    outs = [unpack_out(np.asarray(res.results[ci]["out"]), L) for ci in range(NCORES)]
    return np.concatenate(outs, axis=0), res


def kernel(**inputs) -> np.ndarray:
    out, _ = run(inputs["x"], inputs["k"], inputs["s"], inputs["b"])
    return out
